# revision 26
# baseline (speedup 1.0000x reference)
"""Trainium2 Bass kernel for nn_CICDM_Net (ragged sequence cognitive-diagnosis model).

Strategy (8 NeuronCores, SPMD):
  - The ragged per-(student,concept) softmax over answered exercises is
    factored through per-(student,exercise) aggregates cnt/xsum: softmax
    ratios are shift-invariant, so with G = adj*exp(e2c-10), G2 = exp(e2p-10):
       val    = (xsum@G)/(cnt@G)   on active entries,
       active = (cnt@G) > 0        (exact: all terms nonnegative),
       B      = (xsum@G2)/(cnt@G2)
    and downstream everything is dense matmul.
  - cnt/xsum are built on-device per student as two-level one-hot matmuls:
    e = 32*q + r; cntT_n(q,r) = Bq_n.T @ (m*Ar_n) with Bq=[idx>>5==q],
    Ar=[idx%32==r] built by vector-engine is_equal against an iota tile.
  - Sharding: exercises E sharded 512/core (similarity tables, row-softmaxes,
    output Y columns); students sharded 16/core for the cnt/xsum build.
    One AllToAll moves cnt/xsum to the shard owners; one AllReduce combines
    the per-shard num/den partials.  A is computed replicated.
"""
import sys
import os
import numpy as np

sys.path.insert(0, "/opt/trn_rl_repo")

import concourse.bass as bass
import concourse.tile as tile
import concourse.mybir as mybir
from concourse.vector_clock import ScopedClock
from concourse.bass_utils import run_bass_kernel_spmd

F32 = mybir.dt.float32
F32R = mybir.dt.float32r
BF16 = mybir.dt.bfloat16
AF = mybir.ActivationFunctionType
OP = mybir.AluOpType
AX = mybir.AxisListType

NCORES = 8
E, C, P, H = 4096, 512, 256, 128
N, L = 128, 512
ESH = E // NCORES   # 512 exercises per core
NSH = N // NCORES   # 16 students per core
CT = C // 128       # 4 concept tiles
PT = P // 128       # 2 potential tiles
ET = ESH // 128     # 4 exercise tiles per shard
QL = ESH // 32      # 16 q values per shard

DEBUG = bool(int(os.environ.get("CICDM_DEBUG", "0")))


def _patched_drain_and_barrier(self, tick_clock, wait_clock):
    # Walrus in this container rejects InstDrain carrying sem waits ("Too many
    # sync wait commands").  Put each wait on its own sync-engine NOP before a
    # bare drain, and use the sem-only (EVSEM) all-engine barrier.
    nc = self.nc
    carrier = nc.sync.nop(nofuse=True)
    wait_clock.add_sem_waits(carrier.ins, ScopedClock({None: tick_clock.global_clock}))
    si = carrier.ins.sync_info
    waits = list(si.on_wait) if si is not None else []
    if si is not None:
        si.on_wait = waits[:1]
    for w in waits[1:]:
        n = nc.sync.nop(nofuse=True)
        n.ins.sync_info = mybir.SyncInfo(on_wait=[w], on_update=[])
    nc.sync.drain()
    nc.all_engine_barrier(sem_only=True)
    popped = nc._tile_sem_poison_stack.pop()
    assert popped is self._sem_poison
    nc.clear_and_free_semaphores(list(self.sems.allocated().values()))
    nc.all_engine_barrier(sem_only=True)


tile.TileContext._drain_and_barrier = _patched_drain_and_barrier

_WSPLIT_ID = [0]


def _split_excess_waits(nc):
    # Walrus here accepts at most ONE sync wait per instruction.  Move each
    # excess wait onto its own same-engine NOP placed just before the
    # instruction (per-engine streams are in order, so semantics unchanged).
    for fn in nc.m.functions:
        for bb in fn.blocks:
            new = []
            changed = False
            for ins in bb.instructions:
                si = ins.sync_info
                if si is not None and len(si.on_wait) > 1:
                    waits = list(si.on_wait)
                    si.on_wait = waits[-1:]
                    for w in waits[:-1]:
                        _WSPLIT_ID[0] += 1
                        nop = mybir.InstNoOp(
                            name=f"wsplit-{_WSPLIT_ID[0]}",
                            engine=ins.engine, ins=[], outs=[],
                        )
                        nop.sync_info = mybir.SyncInfo(on_wait=[w], on_update=[])
                        new.append(nop)
                    changed = True
                new.append(ins)
            if changed:
                bb.instructions = new


def build_nc():
    nc = bass.Bass()
    din = lambda name, shape: nc.declare_dram_parameter(name, shape, F32, isOutput=False)
    exer = din("exer_sh", [ESH, H])
    conc = din("conc", [C, H])
    pote = din("pote", [P, H])
    adj = din("adj_sh", [ESH, C])
    qb = nc.declare_dram_parameter("q_bf", [NSH, L], BF16, isOutput=False)
    rb = nc.declare_dram_parameter("r_bf", [NSH, L], BF16, isOutput=False)
    mb = nc.declare_dram_parameter("mask_bf", [NSH, L], BF16, isOutput=False)
    sb_in = nc.declare_dram_parameter("scores_bf", [NSH, L], BF16, isOutput=False)
    lamc = din("lambd_col", [128, ET])
    slr = din("slide_sh", [1, ESH])
    gur = din("guess_sh", [1, ESH])
    iotab = nc.declare_dram_parameter("iota_bf", [128, 128], BF16, isOutput=False)
    ident = din("ident", [128, 128])
    a_out = nc.declare_dram_parameter("A_out", [N, C], F32, isOutput=True)
    y_out = nc.declare_dram_parameter("Y_sh", [N, ESH], F32, isOutput=True)
    if DEBUG:
        nd_out = nc.declare_dram_parameter("dbg_nd", [N, 1536], F32, isOutput=True)
        pay_out = nc.declare_dram_parameter("dbg_pay", [NSH * 128, 64], F32, isOutput=True)
        recv_out = nc.declare_dram_parameter("dbg_recv", [128, 1024], F32, isOutput=True)
        pay5_out = nc.declare_dram_parameter("dbg_pay5", [128, 1024], F32, isOutput=True)
        g_out = nc.declare_dram_parameter("dbg_G", [128, ET * C], F32, isOutput=True)
        g2_out = nc.declare_dram_parameter("dbg_G2", [128, ET * P], F32, isOutput=True)

    with tile.TileContext(nc) as tc:
        with (
            tc.tile_pool(name="const", bufs=1) as cpool,
            tc.tile_pool(name="persist", bufs=1) as pp,
            tc.tile_pool(name="work", bufs=3) as wp,
            tc.tile_pool(name="bwork", bufs=4) as bw,
            tc.tile_pool(name="dram", bufs=1, space="DRAM") as dp,
        ):
            tpp_cm = tc.tile_pool(name="tpp", bufs=2, space="PSUM")
            tpp = tpp_cm.__enter__()
            mmp_cm = tc.tile_pool(name="mmp", bufs=2, space="PSUM")
            mmp = mmp_cm.__enter__()
            psB_cm = tc.tile_pool(name="psB", bufs=2, space="PSUM")
            psB = psB_cm.__enter__()
            iob = cpool.tile([128, 128], BF16, tag="iota_bf")
            nc.sync.dma_start(iob[:], iotab[:])
            idn = cpool.tile([128, 128], F32, tag="ident")
            nc.sync.dma_start(idn[:], ident[:])
            neg10 = cpool.tile([128, 1], F32, tag="neg10")
            nc.vector.memset(neg10[:], -10.0)

            # ---------------- Phase B: cnt/xsum via two-level one-hot ----------
            # per-(student, l-tile) columns via DMA-transpose: (16, 128)
            # blocks of the (NSH, L) bf16 inputs -> SBUF (128, lt, n)
            def load_nl(dram_t, tag, eng):
                t = bw.tile([128, 4, NSH], BF16, tag=tag)
                for k in range(4):
                    eng.dma_start(t[:, k, :], dram_t[:, 128 * k : 128 * (k + 1)],
                                  transpose=True)
                return t

            q_sbh = load_nl(qb, "q_sbh", nc.scalar)
            r_sbh = load_nl(rb, "r_sbh", nc.scalar)
            msk_sbh = load_nl(mb, "msk_sbh", nc.scalar)
            sco_sbh = load_nl(sb_in, "sco_sbh", nc.scalar)

            # tensor_scalar scalar operands must be f32: cast the columns up
            def up32(t, tag):
                o = bw.tile([128, 4, NSH], F32, tag=tag)
                nc.vector.tensor_copy(o[:], t[:])
                return o

            q_sb = up32(q_sbh, "q_sb")
            r_sb = up32(r_sbh, "r_sb")
            msk_sb = up32(msk_sbh, "msk_sb")
            sco_sb = up32(sco_sbh, "sco_sb")
            xm_sb = bw.tile([128, 4, NSH], F32, tag="xm_sb")
            nc.vector.tensor_mul(xm_sb[:], msk_sb[:], sco_sb[:])

            # payload [dest_core][n_loc][t][q_loc][r]
            pay5 = dp.tile([NCORES, NSH, 2, QL, 32], F32, tag="pay")
            sb_all = pp.tile([128, NSH, 64], F32, tag="sb_all")
            for n in range(NSH):
                ps = psB.tile([128, 64], F32, tag="bq_ps")
                for lt in range(4):
                    qcol = q_sb[:, lt, n : n + 1]
                    rcol = r_sb[:, lt, n : n + 1]
                    bq = bw.tile([128, 128], BF16, tag="bq")
                    nc.vector.tensor_scalar(bq[:], iob[:], qcol, None, OP.is_equal)
                    rhs = bw.tile([128, 64], BF16, tag="rhs")
                    nc.vector.tensor_scalar(
                        rhs[:, 0:32], iob[:, 0:32], rcol,
                        msk_sb[:, lt, n : n + 1], OP.is_equal, OP.mult,
                    )
                    nc.vector.tensor_scalar(
                        rhs[:, 32:64], iob[:, 0:32], rcol,
                        xm_sb[:, lt, n : n + 1], OP.is_equal, OP.mult,
                    )
                    nc.tensor.matmul(ps[:], bq[:], rhs[:], start=(lt == 0), stop=(lt == 3))
                nc.vector.tensor_copy(sb_all[:, n, :], ps[:])
                if DEBUG:
                    nc.sync.dma_start(pay_out[128 * n : 128 * (n + 1), :], sb_all[:, n, :])
            # one DMA per (dest, table): 16 partitions x (n, r)
            for dch in range(NCORES):
                for t in range(2):
                    src_ap = bass.AP(
                        sb_all[:].tensor, 16 * dch * (NSH * 64) + 32 * t,
                        [[NSH * 64, QL], [64, NSH], [1, 32]],
                    )
                    nc.sync.dma_start(pay5[dch, :, t, :, :].rearrange(
                        "n q r -> q n r"), src_ap)

            # AllToAll: recv [src_core][n_loc][t][q_loc][r], q = 16*me + q_loc
            recv5 = dp.tile([NCORES, NSH, 2, QL, 32], F32, tag="recv")
            nc.gpsimd.collective_compute(
                "AllToAll", OP.bypass,
                replica_groups=[list(range(NCORES))],
                ins=[pay5.opt()], outs=[recv5.opt()],
            )
            psB_cm.__exit__(None, None, None)

            # ---------------- Phase A: similarity tables (E-shard) -------------
            def norm_rows(dram_t, rows, tag):
                tiles = []
                for i in range(rows // 128):
                    t = pp.tile([128, H], F32, tag=f"{tag}{i}")
                    nc.sync.dma_start(t[:], dram_t[128 * i : 128 * (i + 1), :])
                    sq = wp.tile([128, H], F32, tag="sq_scratch")
                    nsq = wp.tile([128, 1], F32, tag="nsq")
                    nc.scalar.activation(sq[:], t[:], AF.Square, accum_out=nsq[:])
                    sr = wp.tile([128, 1], F32, tag="sr")
                    nc.scalar.activation(sr[:], nsq[:], AF.Sqrt)
                    rn = wp.tile([128, 1], F32, tag="rn")
                    nc.vector.reciprocal(rn[:], sr[:])
                    nc.vector.tensor_scalar(t[:], t[:], rn[:], None, OP.mult)
                    tiles.append(t)
                return tiles

            exn = norm_rows(exer, ESH, "exn")
            con = norm_rows(conc, C, "con")
            pon = norm_rows(pote, P, "pon")

            def transpose_to(dst_ap, src_ap):
                tp = tpp.tile([128, 128], F32, tag="tp")
                nc.tensor.transpose(tp[:], src_ap, idn[:])
                nc.vector.tensor_copy(dst_ap, tp[:])

            exnT = pp.tile([128, ET, 128], F32, tag="exnT")   # (H, e)
            for i in range(ET):
                transpose_to(exnT[:, i, :], exn[i][:])
            conT = pp.tile([128, CT, 128], F32, tag="conT")   # (H, c)
            for i in range(CT):
                transpose_to(conT[:, i, :], con[i][:])
            ponT = pp.tile([128, PT, 128], F32, tag="ponT")   # (H, p)
            for i in range(PT):
                transpose_to(ponT[:, i, :], pon[i][:])

            # sigmoid gate columns (128, ET) indexed by local e
            lam_raw = wp.tile([128, ET], F32, tag="lam_raw")
            nc.sync.dma_start(lam_raw[:], lamc[:])
            lam_s = pp.tile([128, ET], F32, tag="lam_sig")
            nc.scalar.activation(lam_s[:], lam_raw[:], AF.Sigmoid)
            c1 = pp.tile([128, ET], F32, tag="c1")  # 1 - lam
            nc.vector.tensor_scalar(c1[:], lam_s[:], -1.0, 1.0, OP.mult, OP.add)

            # row (1, ESH) versions of the output gates for the Y combine
            def load_gate_row(dram_t, tag):
                t = wp.tile([1, ESH], F32, tag="gate_row_raw")
                nc.sync.dma_start(t[:], dram_t[:])
                s = pp.tile([1, ESH], F32, tag=f"{tag}_rsig")
                nc.scalar.activation(s[:], t[:], AF.Sigmoid)
                return s

            gu_row_f = load_gate_row(gur, "gu")
            sl_row = load_gate_row(slr, "sl")
            c2_row_f = pp.tile([1, ESH], F32, tag="c2_row_f")  # 1 - sl - gu
            nc.vector.tensor_add(c2_row_f[:], sl_row[:], gu_row_f[:])
            nc.vector.tensor_scalar(c2_row_f[:], c2_row_f[:], -1.0, 1.0, OP.mult, OP.add)
            gu_row = pp.tile([1, ESH], F32R, tag="gu_row")
            nc.vector.tensor_copy(gu_row[:], gu_row_f[:])
            c2_row = pp.tile([1, ESH], F32R, tag="c2_row")
            nc.vector.tensor_copy(c2_row[:], c2_row_f[:])
            ones_row_f = cpool.tile([1, 128], F32, tag="ones_row_f")
            nc.vector.memset(ones_row_f[:], 1.0)
            ones_row = cpool.tile([1, 128], F32R, tag="ones_row")
            nc.vector.tensor_copy(ones_row[:], ones_row_f[:])

            # e2c shard -> G (e,c); SM1s^T (c,e) scaled by (1-lam)/rowsum
            G = pp.tile([128, ET, C], F32R, tag="G")
            sm1T = pp.tile([128, CT, ESH], F32R, tag="sm1T")
            adj_sb = pp.tile([128, ET, C], F32, tag="adj_sb")
            for i in range(ET):
                nc.sync.dma_start(adj_sb[:, i, :], adj[128 * i : 128 * (i + 1), :])
            for i in range(ET):
                mm = mmp.tile([128, C], F32, tag="mm")
                for kc in range(CT):
                    nc.tensor.matmul(
                        mm[:, 128 * kc : 128 * (kc + 1)],
                        exnT[:, i, :], conT[:, kc, :], start=True, stop=True,
                    )
                ex = wp.tile([128, C], F32, tag="expA")
                nc.scalar.activation(ex[:], mm[:], AF.Exp, bias=neg10[:, 0:1], scale=10.0)
                nc.vector.tensor_mul(G[:, i, :], ex[:], adj_sb[:, i, :])
                rs = wp.tile([128, 1], F32, tag="rsA")
                nc.vector.reduce_sum(rs[:], G[:, i, :].bitcast(F32), axis=AX.X)
                rr = wp.tile([128, 1], F32, tag="rrA")
                nc.vector.reciprocal(rr[:], rs[:])
                cc = wp.tile([128, 1], F32, tag="ccA")
                nc.vector.tensor_scalar(cc[:], rr[:], c1[:, i : i + 1], None, OP.mult)
                s1 = wp.tile([128, C], F32, tag="sm1s")
                nc.scalar.activation(s1[:], G[:, i, :].bitcast(F32), AF.Identity, scale=cc[:])
                for cb in range(CT):
                    transpose_to(sm1T[:, cb, 128 * i : 128 * (i + 1)],
                                 s1[:, 128 * cb : 128 * (cb + 1)])

            # e2p shard -> G2 (e,p); SM2s^T (p,e) scaled by lam/rowsum
            G2 = pp.tile([128, ET, P], F32R, tag="G2")
            sm2T = pp.tile([128, PT, ESH], F32R, tag="sm2T")
            for i in range(ET):
                mm = mmp.tile([128, C], F32, tag="mm")
                for kp in range(PT):
                    nc.tensor.matmul(
                        mm[:, 128 * kp : 128 * (kp + 1)],
                        exnT[:, i, :], ponT[:, kp, :], start=True, stop=True,
                    )
                nc.scalar.activation(G2[:, i, :], mm[:, 0:P], AF.Exp, bias=neg10[:, 0:1], scale=10.0)
                rs = wp.tile([128, 1], F32, tag="rsP")
                nc.vector.reduce_sum(rs[:], G2[:, i, :].bitcast(F32), axis=AX.X)
                rr = wp.tile([128, 1], F32, tag="rrP")
                nc.vector.reciprocal(rr[:], rs[:])
                cc = wp.tile([128, 1], F32, tag="ccP")
                nc.vector.tensor_scalar(cc[:], rr[:], lam_s[:, i : i + 1], None, OP.mult)
                s2 = wp.tile([128, P], F32, tag="sm2s")
                nc.scalar.activation(s2[:], G2[:, i, :].bitcast(F32), AF.Identity, scale=cc[:])
                for pb in range(PT):
                    transpose_to(sm2T[:, pb, 128 * i : 128 * (i + 1)],
                                 s2[:, 128 * pb : 128 * (pb + 1)])

            # c2c -> Eexp (c', c), replicated
            Eexp = pp.tile([128, CT, C], F32R, tag="Eexp")
            for i in range(CT):
                mm = mmp.tile([128, C], F32, tag="mm")
                for kc in range(CT):
                    nc.tensor.matmul(
                        mm[:, 128 * kc : 128 * (kc + 1)],
                        conT[:, i, :], conT[:, kc, :], start=True, stop=True,
                    )
                nc.scalar.activation(Eexp[:, i, :], mm[:], AF.Exp, bias=neg10[:, 0:1], scale=10.0)

            mmp_cm.__exit__(None, None, None)

            # ---------------- Phase C: shard partials of xsum/cnt @ G, @ G2 ----
            psC_cm = tc.tile_pool(name="psC", bufs=1, space="PSUM")
            psC = psC_cm.__enter__()
            num = psC.tile([128, C], F32, tag="num")
            den = psC.tile([128, C], F32, tag="den")
            numB = psC.tile([128, P], F32, tag="numB")
            denB = psC.tile([128, P], F32, tag="denB")
            # load received tables with students on partitions, then
            # transpose each (t, i) block on the PE to lhsT layout
            rsb = pp.tile([128, 2, QL, 32], F32, tag="rsb")
            nc.sync.dma_start(
                rsb[:], bass.AP(recv5[:].tensor, 0, [[1024, 128], [1, 1024]])
            )
            if DEBUG:
                p5 = pp.tile([128, 1024], F32, tag="p5dump")
                nc.sync.dma_start(
                    p5[:], bass.AP(pay5[:].tensor, 0, [[1024, 128], [1, 1024]])
                )
                nc.sync.dma_start(pay5_out[:], p5[:])
                nc.sync.dma_start(recv_out[:], rsb[:])
                nc.sync.dma_start(g_out[:], G.rearrange("p a b -> p (a b)").bitcast(F32))
                nc.sync.dma_start(g2_out[:], G2.rearrange("p a b -> p (a b)").bitcast(F32))
            for i in range(ET):
                cl = bw.tile([128, 128], F32R, tag="lh_cnt")
                xl = bw.tile([128, 128], F32R, tag="lh_xs")
                transpose_to(cl[:], rsb[:, 0, 4 * i : 4 * i + 4, :])
                transpose_to(xl[:], rsb[:, 1, 4 * i : 4 * i + 4, :])
                st, sp = (i == 0), (i == ET - 1)
                nc.tensor.matmul(den[:], cl[:], G[:, i, :], start=st, stop=sp)
                nc.tensor.matmul(num[:], xl[:], G[:, i, :], start=st, stop=sp)
                nc.tensor.matmul(denB[:], cl[:], G2[:, i, :], start=st, stop=sp)
                nc.tensor.matmul(numB[:], xl[:], G2[:, i, :], start=st, stop=sp)

            arstB = wp.tile([128, 512], F32, tag="arstB")
            nc.vector.tensor_copy(arstB[:, 0:256], numB[:])
            nc.vector.tensor_copy(arstB[:, 256:512], denB[:])
            arstA = wp.tile([128, 1024], F32, tag="arstA")
            nc.vector.tensor_copy(arstA[:, 0:512], num[:])
            nc.vector.tensor_copy(arstA[:, 512:1024], den[:])
            psC_cm.__exit__(None, None, None)
            arinB = dp.tile([128, 512], F32, tag="arinB")
            aroutB = dp.tile([128, 512], F32, tag="aroutB")
            nc.sync.dma_start(arinB[:], arstB[:])
            nc.gpsimd.collective_compute(
                "AllReduce", OP.add,
                replica_groups=[list(range(NCORES))],
                ins=[arinB.opt()], outs=[aroutB.opt()],
            )
            arinA = dp.tile([128, 1024], F32, tag="arinA")
            aroutA = dp.tile([128, 1024], F32, tag="aroutA")
            nc.sync.dma_start(arinA[:], arstA[:])
            nc.gpsimd.collective_compute(
                "AllReduce", OP.add,
                replica_groups=[list(range(NCORES))],
                ins=[arinA.opt()], outs=[aroutA.opt()],
            )
            ndB = pp.tile([128, 512], F32, tag="ndB")
            nc.scalar.dma_start(ndB[:], aroutB[:])
            nd = pp.tile([128, 1024], F32, tag="nd")
            nc.sync.dma_start(nd[:], aroutA[:])
            numf, denf = nd[:, 0:512], nd[:, 512:1024]
            numBf, denBf = ndB[:, 0:256], ndB[:, 256:512]
            if DEBUG:
                nc.sync.dma_start(nd_out[:, 0:1024], nd[:])
                nc.sync.dma_start(nd_out[:, 1024:1536], ndB[:])

            # ---------------- Phase D: B branch first (overlaps AR-A) ---------
            rdB = wp.tile([128, P], F32, tag="rdB")
            nc.vector.reciprocal(rdB[:], denBf)
            B_sb = pp.tile([128, P], F32, tag="B_sb")
            nc.vector.tensor_mul(B_sb[:], numBf, rdB[:])
            BT = pp.tile([128, PT, 128], F32R, tag="BT")
            for pb in range(PT):
                transpose_to(BT[:, pb, :], B_sb[:, 128 * pb : 128 * (pb + 1)])

            psE_cm = tc.tile_pool(name="psE", bufs=1, space="PSUM")
            psE = psE_cm.__enter__()
            yps = psE.tile([128, ESH], F32, tag="yps")
            for kp in range(PT):
                nc.tensor.matmul(yps[:], BT[:, kp, :], sm2T[:, kp, :],
                                 start=(kp == 0), stop=False)
            c2b = psE.tile([128, ESH], F32, tag="c2b")
            nc.tensor.matmul(c2b[:], ones_row[:], c2_row[:], start=True, stop=True)
            gub = psE.tile([128, ESH], F32, tag="gub")
            nc.tensor.matmul(gub[:], ones_row[:], gu_row[:], start=True, stop=True)

            # ---------------- A branch ----------------------------------------
            act = pp.tile([128, C], F32, tag="act")
            nc.vector.tensor_scalar(act[:], denf, 0.0, None, OP.is_gt)
            oma = wp.tile([128, C], F32, tag="oma")
            nc.vector.tensor_scalar(oma[:], act[:], -1.0, 1.0, OP.mult, OP.add)
            nc.vector.tensor_add(oma[:], denf, oma[:])       # den + (1-active)
            rden = wp.tile([128, C], F32, tag="rden")
            nc.vector.reciprocal(rden[:], oma[:])
            va = pp.tile([128, C], F32, tag="va")
            nc.vector.tensor_mul(va[:], numf, act[:])
            nc.vector.tensor_mul(va[:], va[:], rden[:])

            vaT = pp.tile([128, CT, 128], F32R, tag="vaT")
            actT = pp.tile([128, CT, 128], F32R, tag="actT")
            for cb in range(CT):
                transpose_to(vaT[:, cb, :], va[:, 128 * cb : 128 * (cb + 1)])
                transpose_to(actT[:, cb, :], act[:, 128 * cb : 128 * (cb + 1)])

            psD_cm = tc.tile_pool(name="psD", bufs=1, space="PSUM")
            psD = psD_cm.__enter__()
            numA = psD.tile([128, C], F32, tag="numA")
            denA = psD.tile([128, C], F32, tag="denA")
            for kc in range(CT):
                st, sp = (kc == 0), (kc == CT - 1)
                nc.tensor.matmul(numA[:], vaT[:, kc, :], Eexp[:, kc, :], start=st, stop=sp)
                nc.tensor.matmul(denA[:], actT[:, kc, :], Eexp[:, kc, :], start=st, stop=sp)
            rdA = wp.tile([128, C], F32, tag="rdA")
            nc.vector.reciprocal(rdA[:], denA[:])
            A_sb = pp.tile([128, C], F32, tag="A_sb")
            nc.vector.tensor_mul(A_sb[:], numA[:], rdA[:])
            nc.sync.dma_start(a_out[:], A_sb[:])
            AT = pp.tile([128, CT, 128], F32R, tag="AT")
            for cb in range(CT):
                transpose_to(AT[:, cb, :], A_sb[:, 128 * cb : 128 * (cb + 1)])
            psD_cm.__exit__(None, None, None)

            # ---------------- Phase E: finish Y = YB + YA ----------------------
            for kc in range(CT):
                nc.tensor.matmul(yps[:], AT[:, kc, :], sm1T[:, kc, :],
                                 start=False, stop=(kc == CT - 1))
            yc = pp.tile([128, ESH], F32, tag="yc")
            nc.vector.tensor_scalar(yc[:], yps[:], 1e-8, 1.0, OP.max, OP.min)
            nc.vector.tensor_mul(yc[:], yc[:], c2b[:])
            nc.vector.tensor_add(yc[:], yc[:], gub[:])
            nc.sync.dma_start(y_out[:], yc[:])
            psE_cm.__exit__(None, None, None)
            tpp_cm.__exit__(None, None, None)

    _split_excess_waits(nc)
    return nc


_NC = None


def _get_nc():
    global _NC
    if _NC is None:
        _NC = build_nc()
    return _NC


def kernel(exer_idx, mask, scores, exer_matrix, conc_matrix, pote_matrix,
           lambd, guess, slide, adj):
    exer_idx = np.asarray(exer_idx)
    mask = np.asarray(mask)
    scores = np.asarray(scores, dtype=np.float32)
    exer_matrix = np.asarray(exer_matrix, dtype=np.float32)
    conc_matrix = np.asarray(conc_matrix, dtype=np.float32)
    pote_matrix = np.asarray(pote_matrix, dtype=np.float32)
    lambd = np.asarray(lambd, dtype=np.float32)
    guess = np.asarray(guess, dtype=np.float32)
    slide = np.asarray(slide, dtype=np.float32)
    adj = np.asarray(adj, dtype=np.float32)

    import ml_dtypes
    iota_bf = np.broadcast_to(
        np.arange(128, dtype=np.float32), (128, 128)).astype(ml_dtypes.bfloat16)
    ident = np.eye(128, dtype=np.float32)

    in_maps = []
    for c in range(NCORES):
        es = slice(ESH * c, ESH * (c + 1))
        ns = slice(NSH * c, NSH * (c + 1))
        in_maps.append({
            "exer_sh": np.ascontiguousarray(exer_matrix[es]),
            "conc": conc_matrix,
            "pote": pote_matrix,
            "adj_sh": np.ascontiguousarray(adj[es]),
            "q_bf": (exer_idx[ns] >> 5).astype(ml_dtypes.bfloat16),
            "r_bf": (exer_idx[ns] & 31).astype(ml_dtypes.bfloat16),
            "mask_bf": mask[ns].astype(ml_dtypes.bfloat16),
            "scores_bf": scores[ns].astype(ml_dtypes.bfloat16),
            "lambd_col": np.ascontiguousarray(
                lambd[0, es].reshape(ET, 128).T),
            "slide_sh": np.ascontiguousarray(slide[:, es]),
            "guess_sh": np.ascontiguousarray(guess[:, es]),
            "iota_bf": iota_bf,
            "ident": ident,
        })

    res = run_bass_kernel_spmd(_get_nc(), in_maps, list(range(NCORES)))
    A = res.results[0]["A_out"]
    Y = np.concatenate([res.results[c]["Y_sh"] for c in range(NCORES)], axis=1)
    if DEBUG:
        kernel.debug_results = res.results
    return A, Y


# revision 27
# speedup vs baseline: 1.1406x; 1.1406x over previous
"""Trainium2 Bass kernel for nn_CICDM_Net (ragged sequence cognitive-diagnosis model).

Strategy (8 NeuronCores, SPMD):
  - The ragged per-(student,concept) softmax over answered exercises is
    factored through per-(student,exercise) aggregates cnt/xsum: softmax
    ratios are shift-invariant, so with G = adj*exp(e2c-10), G2 = exp(e2p-10):
       val    = (xsum@G)/(cnt@G)   on active entries,
       active = (cnt@G) > 0        (exact: all terms nonnegative),
       B      = (xsum@G2)/(cnt@G2)
    and downstream everything is dense matmul.
  - cnt/xsum are built on-device per student as two-level one-hot matmuls:
    e = 32*q + r; cntT_n(q,r) = Bq_n.T @ (m*Ar_n) with Bq=[idx>>5==q],
    Ar=[idx%32==r] built by vector-engine is_equal against an iota tile.
  - Sharding: exercises E sharded 512/core (similarity tables, row-softmaxes,
    output Y columns); students sharded 16/core for the cnt/xsum build.
    One AllToAll moves cnt/xsum to the shard owners; one AllReduce combines
    the per-shard num/den partials.  A is computed replicated.
"""
import sys
import os
import numpy as np

sys.path.insert(0, "/opt/trn_rl_repo")

import concourse.bass as bass
import concourse.tile as tile
import concourse.mybir as mybir
from concourse.vector_clock import ScopedClock
from concourse.bass_utils import run_bass_kernel_spmd

F32 = mybir.dt.float32
F32R = mybir.dt.float32r
BF16 = mybir.dt.bfloat16
AF = mybir.ActivationFunctionType
OP = mybir.AluOpType
AX = mybir.AxisListType

NCORES = 8
E, C, P, H = 4096, 512, 256, 128
N, L = 128, 512
ESH = E // NCORES   # 512 exercises per core
NSH = N // NCORES   # 16 students per core
CT = C // 128       # 4 concept tiles
PT = P // 128       # 2 potential tiles
ET = ESH // 128     # 4 exercise tiles per shard
QL = ESH // 32      # 16 q values per shard

DEBUG = bool(int(os.environ.get("CICDM_DEBUG", "0")))


def _patched_drain_and_barrier(self, tick_clock, wait_clock):
    # Walrus in this container rejects InstDrain carrying sem waits ("Too many
    # sync wait commands").  Put each wait on its own sync-engine NOP before a
    # bare drain, and use the sem-only (EVSEM) all-engine barrier.
    nc = self.nc
    carrier = nc.sync.nop(nofuse=True)
    wait_clock.add_sem_waits(carrier.ins, ScopedClock({None: tick_clock.global_clock}))
    si = carrier.ins.sync_info
    waits = list(si.on_wait) if si is not None else []
    if si is not None:
        si.on_wait = waits[:1]
    for w in waits[1:]:
        n = nc.sync.nop(nofuse=True)
        n.ins.sync_info = mybir.SyncInfo(on_wait=[w], on_update=[])
    nc.sync.drain()
    nc.all_engine_barrier(sem_only=True)
    popped = nc._tile_sem_poison_stack.pop()
    assert popped is self._sem_poison
    nc.clear_and_free_semaphores(list(self.sems.allocated().values()))
    nc.all_engine_barrier(sem_only=True)


tile.TileContext._drain_and_barrier = _patched_drain_and_barrier

_WSPLIT_ID = [0]


def _split_excess_waits(nc):
    # Walrus here accepts at most ONE sync wait per instruction.  Move each
    # excess wait onto its own same-engine NOP placed just before the
    # instruction (per-engine streams are in order, so semantics unchanged).
    for fn in nc.m.functions:
        for bb in fn.blocks:
            new = []
            changed = False
            for ins in bb.instructions:
                si = ins.sync_info
                if si is not None and len(si.on_wait) > 1:
                    waits = list(si.on_wait)
                    si.on_wait = waits[-1:]
                    for w in waits[:-1]:
                        _WSPLIT_ID[0] += 1
                        nop = mybir.InstNoOp(
                            name=f"wsplit-{_WSPLIT_ID[0]}",
                            engine=ins.engine, ins=[], outs=[],
                        )
                        nop.sync_info = mybir.SyncInfo(on_wait=[w], on_update=[])
                        new.append(nop)
                    changed = True
                new.append(ins)
            if changed:
                bb.instructions = new


def build_nc():
    nc = bass.Bass()
    din = lambda name, shape: nc.declare_dram_parameter(name, shape, F32, isOutput=False)
    exer = din("exer_sh", [ESH, H])
    conc = din("conc", [C, H])
    pote = din("pote", [P, H])
    adj = din("adj_sh", [ESH, C])
    qb = nc.declare_dram_parameter("q_bf", [NSH, L], BF16, isOutput=False)
    rb = nc.declare_dram_parameter("r_bf", [NSH, L], BF16, isOutput=False)
    mb = nc.declare_dram_parameter("mask_bf", [NSH, L], BF16, isOutput=False)
    sb_in = nc.declare_dram_parameter("scores_bf", [NSH, L], BF16, isOutput=False)
    lamc = din("lambd_col", [128, ET])
    slr = din("slide_sh", [1, ESH])
    gur = din("guess_sh", [1, ESH])
    iotab = nc.declare_dram_parameter("iota_bf", [128, 128], BF16, isOutput=False)
    ident = din("ident", [128, 128])
    a_out = nc.declare_dram_parameter("A_out", [N, C], F32, isOutput=True)
    y_out = nc.declare_dram_parameter("Y_sh", [N, ESH], F32, isOutput=True)
    if DEBUG:
        nd_out = nc.declare_dram_parameter("dbg_nd", [N, 1536], F32, isOutput=True)
        pay_out = nc.declare_dram_parameter("dbg_pay", [NSH * 128, 64], F32, isOutput=True)
        recv_out = nc.declare_dram_parameter("dbg_recv", [128, 1024], F32, isOutput=True)
        pay5_out = nc.declare_dram_parameter("dbg_pay5", [128, 1024], F32, isOutput=True)
        g_out = nc.declare_dram_parameter("dbg_G", [128, ET * C], F32, isOutput=True)
        g2_out = nc.declare_dram_parameter("dbg_G2", [128, ET * P], F32, isOutput=True)

    with tile.TileContext(nc) as tc:
        with (
            tc.tile_pool(name="const", bufs=1) as cpool,
            tc.tile_pool(name="persist", bufs=1) as pp,
            tc.tile_pool(name="work", bufs=3) as wp,
            tc.tile_pool(name="bwork", bufs=4) as bw,
            tc.tile_pool(name="dram", bufs=1, space="DRAM") as dp,
        ):
            tpp_cm = tc.tile_pool(name="tpp", bufs=2, space="PSUM")
            tpp = tpp_cm.__enter__()
            mmp_cm = tc.tile_pool(name="mmp", bufs=2, space="PSUM")
            mmp = mmp_cm.__enter__()
            psB_cm = tc.tile_pool(name="psB", bufs=2, space="PSUM")
            psB = psB_cm.__enter__()
            iob = cpool.tile([128, 128], BF16, tag="iota_bf")
            nc.sync.dma_start(iob[:], iotab[:])
            idn = cpool.tile([128, 128], F32, tag="ident")
            nc.sync.dma_start(idn[:], ident[:])
            neg10 = cpool.tile([128, 1], F32, tag="neg10")
            nc.vector.memset(neg10[:], -10.0)

            # ---------------- Phase B: cnt/xsum via two-level one-hot ----------
            # per-(student, l-tile) columns via DMA-transpose: (16, 128)
            # blocks of the (NSH, L) bf16 inputs -> SBUF (128, lt, n)
            def load_nl(dram_t, tag, eng):
                t = bw.tile([128, 4, NSH], BF16, tag=tag)
                for k in range(4):
                    eng.dma_start(t[:, k, :], dram_t[:, 128 * k : 128 * (k + 1)],
                                  transpose=True)
                return t

            q_sbh = load_nl(qb, "q_sbh", nc.scalar)
            r_sbh = load_nl(rb, "r_sbh", nc.scalar)
            msk_sbh = load_nl(mb, "msk_sbh", nc.scalar)
            sco_sbh = load_nl(sb_in, "sco_sbh", nc.scalar)

            # tensor_scalar scalar operands must be f32: cast the columns up
            def up32(t, tag):
                o = bw.tile([128, 4, NSH], F32, tag=tag)
                nc.vector.tensor_copy(o[:], t[:])
                return o

            q_sb = up32(q_sbh, "q_sb")
            r_sb = up32(r_sbh, "r_sb")
            msk_sb = up32(msk_sbh, "msk_sb")
            sco_sb = up32(sco_sbh, "sco_sb")
            xm_sb = bw.tile([128, 4, NSH], F32, tag="xm_sb")
            nc.vector.tensor_mul(xm_sb[:], msk_sb[:], sco_sb[:])

            # payload [dest_core][n_loc][t][q_loc][r]
            pay5 = dp.tile([NCORES, NSH, 2, QL, 32], F32, tag="pay")
            sb_all = pp.tile([128, NSH, 64], F32, tag="sb_all")
            for n in range(NSH):
                ps = psB.tile([128, 64], F32, tag="bq_ps")
                for lt in range(4):
                    qcol = q_sb[:, lt, n : n + 1]
                    rcol = r_sb[:, lt, n : n + 1]
                    bq = bw.tile([128, 128], BF16, tag="bq")
                    nc.vector.tensor_scalar(bq[:], iob[:], qcol, None, OP.is_equal)
                    rhs = bw.tile([128, 64], BF16, tag="rhs")
                    nc.vector.tensor_scalar(
                        rhs[:, 0:32], iob[:, 0:32], rcol,
                        msk_sb[:, lt, n : n + 1], OP.is_equal, OP.mult,
                    )
                    nc.vector.tensor_scalar(
                        rhs[:, 32:64], iob[:, 0:32], rcol,
                        xm_sb[:, lt, n : n + 1], OP.is_equal, OP.mult,
                    )
                    nc.tensor.matmul(ps[:], bq[:], rhs[:], start=(lt == 0), stop=(lt == 3))
                nc.vector.tensor_copy(sb_all[:, n, :], ps[:])
                if DEBUG:
                    nc.sync.dma_start(pay_out[128 * n : 128 * (n + 1), :], sb_all[:, n, :])
            # one DMA per (dest, table): 16 partitions x (n, r)
            for dch in range(NCORES):
                for t in range(2):
                    src_ap = bass.AP(
                        sb_all[:].tensor, 16 * dch * (NSH * 64) + 32 * t,
                        [[NSH * 64, QL], [64, NSH], [1, 32]],
                    )
                    nc.sync.dma_start(pay5[dch, :, t, :, :].rearrange(
                        "n q r -> q n r"), src_ap)

            # AllToAll: recv [src_core][n_loc][t][q_loc][r], q = 16*me + q_loc
            recv5 = dp.tile([NCORES, NSH, 2, QL, 32], F32, tag="recv")
            nc.gpsimd.collective_compute(
                "AllToAll", OP.bypass,
                replica_groups=[list(range(NCORES))],
                ins=[pay5.opt()], outs=[recv5.opt()],
            )
            psB_cm.__exit__(None, None, None)

            # ---------------- Phase A: similarity tables (E-shard) -------------
            def norm_rows(dram_t, rows, tag):
                tiles = []
                for i in range(rows // 128):
                    t = pp.tile([128, H], F32, tag=f"{tag}{i}")
                    nc.sync.dma_start(t[:], dram_t[128 * i : 128 * (i + 1), :])
                    sq = wp.tile([128, H], F32, tag="sq_scratch")
                    nsq = wp.tile([128, 1], F32, tag="nsq")
                    nc.scalar.activation(sq[:], t[:], AF.Square, accum_out=nsq[:])
                    sr = wp.tile([128, 1], F32, tag="sr")
                    nc.scalar.activation(sr[:], nsq[:], AF.Sqrt)
                    rn = wp.tile([128, 1], F32, tag="rn")
                    nc.vector.reciprocal(rn[:], sr[:])
                    nc.vector.tensor_scalar(t[:], t[:], rn[:], None, OP.mult)
                    tiles.append(t)
                return tiles

            exn = norm_rows(exer, ESH, "exn")
            con = norm_rows(conc, C, "con")
            pon = norm_rows(pote, P, "pon")

            def transpose_to(dst_ap, src_ap):
                tp = tpp.tile([128, 128], F32, tag="tp")
                nc.tensor.transpose(tp[:], src_ap, idn[:])
                nc.vector.tensor_copy(dst_ap, tp[:])

            exnT = pp.tile([128, ET, 128], F32, tag="exnT")   # (H, e)
            for i in range(ET):
                transpose_to(exnT[:, i, :], exn[i][:])
            conT = pp.tile([128, CT, 128], F32, tag="conT")   # (H, c)
            for i in range(CT):
                transpose_to(conT[:, i, :], con[i][:])
            ponT = pp.tile([128, PT, 128], F32, tag="ponT")   # (H, p)
            for i in range(PT):
                transpose_to(ponT[:, i, :], pon[i][:])

            # sigmoid gate columns (128, ET) indexed by local e
            lam_raw = wp.tile([128, ET], F32, tag="lam_raw")
            nc.sync.dma_start(lam_raw[:], lamc[:])
            lam_s = pp.tile([128, ET], F32, tag="lam_sig")
            nc.scalar.activation(lam_s[:], lam_raw[:], AF.Sigmoid)
            c1 = pp.tile([128, ET], F32, tag="c1")  # 1 - lam
            nc.vector.tensor_scalar(c1[:], lam_s[:], -1.0, 1.0, OP.mult, OP.add)

            # row (1, ESH) versions of the output gates for the Y combine
            def load_gate_row(dram_t, tag):
                t = wp.tile([1, ESH], F32, tag="gate_row_raw")
                nc.sync.dma_start(t[:], dram_t[:])
                s = pp.tile([1, ESH], F32, tag=f"{tag}_rsig")
                nc.scalar.activation(s[:], t[:], AF.Sigmoid)
                return s

            gu_row_f = load_gate_row(gur, "gu")
            sl_row = load_gate_row(slr, "sl")
            c2_row_f = pp.tile([1, ESH], F32, tag="c2_row_f")  # 1 - sl - gu
            nc.vector.tensor_add(c2_row_f[:], sl_row[:], gu_row_f[:])
            nc.vector.tensor_scalar(c2_row_f[:], c2_row_f[:], -1.0, 1.0, OP.mult, OP.add)
            gu_row = pp.tile([1, ESH], F32R, tag="gu_row")
            nc.vector.tensor_copy(gu_row[:], gu_row_f[:])
            c2_row = pp.tile([1, ESH], F32R, tag="c2_row")
            nc.vector.tensor_copy(c2_row[:], c2_row_f[:])
            ones_row_f = cpool.tile([1, 128], F32, tag="ones_row_f")
            nc.vector.memset(ones_row_f[:], 1.0)
            ones_row = cpool.tile([1, 128], F32R, tag="ones_row")
            nc.vector.tensor_copy(ones_row[:], ones_row_f[:])

            # e2c shard -> G (e,c); SM1s^T (c,e) scaled by (1-lam)/rowsum
            G = pp.tile([128, ET, C], F32R, tag="G")
            sm1T = pp.tile([128, CT, ESH], F32R, tag="sm1T")
            adj_sb = pp.tile([128, ET, C], F32, tag="adj_sb")
            for i in range(ET):
                nc.sync.dma_start(adj_sb[:, i, :], adj[128 * i : 128 * (i + 1), :])
            for i in range(ET):
                mm = mmp.tile([128, C], F32, tag="mm")
                for kc in range(CT):
                    nc.tensor.matmul(
                        mm[:, 128 * kc : 128 * (kc + 1)],
                        exnT[:, i, :], conT[:, kc, :], start=True, stop=True,
                    )
                ex = wp.tile([128, C], F32, tag="expA")
                nc.scalar.activation(ex[:], mm[:], AF.Exp, bias=neg10[:, 0:1], scale=10.0)
                nc.vector.tensor_mul(G[:, i, :], ex[:], adj_sb[:, i, :])
                rs = wp.tile([128, 1], F32, tag="rsA")
                nc.vector.reduce_sum(rs[:], G[:, i, :].bitcast(F32), axis=AX.X)
                rr = wp.tile([128, 1], F32, tag="rrA")
                nc.vector.reciprocal(rr[:], rs[:])
                cc = wp.tile([128, 1], F32, tag="ccA")
                nc.vector.tensor_scalar(cc[:], rr[:], c1[:, i : i + 1], None, OP.mult)
                s1 = wp.tile([128, C], F32, tag="sm1s")
                nc.scalar.activation(s1[:], G[:, i, :].bitcast(F32), AF.Identity, scale=cc[:])
                for cb in range(CT):
                    transpose_to(sm1T[:, cb, 128 * i : 128 * (i + 1)],
                                 s1[:, 128 * cb : 128 * (cb + 1)])

            # e2p shard -> G2 (e,p); SM2s^T (p,e) scaled by lam/rowsum
            G2 = pp.tile([128, ET, P], F32R, tag="G2")
            sm2T = pp.tile([128, PT, ESH], F32R, tag="sm2T")
            for i in range(ET):
                mm = mmp.tile([128, C], F32, tag="mm")
                for kp in range(PT):
                    nc.tensor.matmul(
                        mm[:, 128 * kp : 128 * (kp + 1)],
                        exnT[:, i, :], ponT[:, kp, :], start=True, stop=True,
                    )
                nc.scalar.activation(G2[:, i, :], mm[:, 0:P], AF.Exp, bias=neg10[:, 0:1], scale=10.0)
                rs = wp.tile([128, 1], F32, tag="rsP")
                nc.vector.reduce_sum(rs[:], G2[:, i, :].bitcast(F32), axis=AX.X)
                rr = wp.tile([128, 1], F32, tag="rrP")
                nc.vector.reciprocal(rr[:], rs[:])
                cc = wp.tile([128, 1], F32, tag="ccP")
                nc.vector.tensor_scalar(cc[:], rr[:], lam_s[:, i : i + 1], None, OP.mult)
                s2 = wp.tile([128, P], F32, tag="sm2s")
                nc.scalar.activation(s2[:], G2[:, i, :].bitcast(F32), AF.Identity, scale=cc[:])
                for pb in range(PT):
                    transpose_to(sm2T[:, pb, 128 * i : 128 * (i + 1)],
                                 s2[:, 128 * pb : 128 * (pb + 1)])

            # c2c -> Eexp (c', c), replicated
            Eexp = pp.tile([128, CT, C], F32R, tag="Eexp")
            for i in range(CT):
                mm = mmp.tile([128, C], F32, tag="mm")
                for kc in range(CT):
                    nc.tensor.matmul(
                        mm[:, 128 * kc : 128 * (kc + 1)],
                        conT[:, i, :], conT[:, kc, :], start=True, stop=True,
                    )
                nc.scalar.activation(Eexp[:, i, :], mm[:], AF.Exp, bias=neg10[:, 0:1], scale=10.0)

            mmp_cm.__exit__(None, None, None)

            # ---------------- Phase C: shard partials of xsum/cnt @ G, @ G2 ----
            psC_cm = tc.tile_pool(name="psC", bufs=1, space="PSUM")
            psC = psC_cm.__enter__()
            num = psC.tile([128, C], F32, tag="num")
            den = psC.tile([128, C], F32, tag="den")
            numB = psC.tile([128, P], F32, tag="numB")
            denB = psC.tile([128, P], F32, tag="denB")
            # load received tables with students on partitions, then
            # transpose each (t, i) block on the PE to lhsT layout
            rsb = pp.tile([128, 2, QL, 32], F32, tag="rsb")
            nc.sync.dma_start(
                rsb[:], bass.AP(recv5[:].tensor, 0, [[1024, 128], [1, 1024]])
            )
            if DEBUG:
                p5 = pp.tile([128, 1024], F32, tag="p5dump")
                nc.sync.dma_start(
                    p5[:], bass.AP(pay5[:].tensor, 0, [[1024, 128], [1, 1024]])
                )
                nc.sync.dma_start(pay5_out[:], p5[:])
                nc.sync.dma_start(recv_out[:], rsb[:])
                nc.sync.dma_start(g_out[:], G.rearrange("p a b -> p (a b)").bitcast(F32))
                nc.sync.dma_start(g2_out[:], G2.rearrange("p a b -> p (a b)").bitcast(F32))
            for i in range(ET):
                cl = bw.tile([128, 128], F32R, tag="lh_cnt")
                xl = bw.tile([128, 128], F32R, tag="lh_xs")
                transpose_to(cl[:], rsb[:, 0, 4 * i : 4 * i + 4, :])
                transpose_to(xl[:], rsb[:, 1, 4 * i : 4 * i + 4, :])
                st, sp = (i == 0), (i == ET - 1)
                nc.tensor.matmul(den[:], cl[:], G[:, i, :], start=st, stop=sp)
                nc.tensor.matmul(num[:], xl[:], G[:, i, :], start=st, stop=sp)
                nc.tensor.matmul(denB[:], cl[:], G2[:, i, :], start=st, stop=sp)
                nc.tensor.matmul(numB[:], xl[:], G2[:, i, :], start=st, stop=sp)

            arst = wp.tile([128, 1536], F32, tag="arst")
            nc.vector.tensor_copy(arst[:, 1024:1280], numB[:])
            nc.vector.tensor_copy(arst[:, 1280:1536], denB[:])
            nc.vector.tensor_copy(arst[:, 0:512], num[:])
            nc.vector.tensor_copy(arst[:, 512:1024], den[:])
            psC_cm.__exit__(None, None, None)
            arin = dp.tile([128, 1536], F32, tag="arin")
            arout = dp.tile([128, 1536], F32, tag="arout")
            nc.sync.dma_start(arin[:], arst[:])
            nc.gpsimd.collective_compute(
                "AllReduce", OP.add,
                replica_groups=[list(range(NCORES))],
                ins=[arin.opt()], outs=[arout.opt()],
            )
            nd = pp.tile([128, 1536], F32, tag="nd")
            nc.sync.dma_start(nd[:], arout[:])
            numf, denf = nd[:, 0:512], nd[:, 512:1024]
            numBf, denBf = nd[:, 1024:1280], nd[:, 1280:1536]
            if DEBUG:
                nc.sync.dma_start(nd_out[:], nd[:])

            # ---------------- Phase D: B branch first (overlaps AR-A) ---------
            rdB = wp.tile([128, P], F32, tag="rdB")
            nc.vector.reciprocal(rdB[:], denBf)
            B_sb = pp.tile([128, P], F32, tag="B_sb")
            nc.vector.tensor_mul(B_sb[:], numBf, rdB[:])
            BT = pp.tile([128, PT, 128], F32R, tag="BT")
            for pb in range(PT):
                transpose_to(BT[:, pb, :], B_sb[:, 128 * pb : 128 * (pb + 1)])

            psE_cm = tc.tile_pool(name="psE", bufs=1, space="PSUM")
            psE = psE_cm.__enter__()
            yps = psE.tile([128, ESH], F32, tag="yps")
            for kp in range(PT):
                nc.tensor.matmul(yps[:], BT[:, kp, :], sm2T[:, kp, :],
                                 start=(kp == 0), stop=False)
            c2b = psE.tile([128, ESH], F32, tag="c2b")
            nc.tensor.matmul(c2b[:], ones_row[:], c2_row[:], start=True, stop=True)
            gub = psE.tile([128, ESH], F32, tag="gub")
            nc.tensor.matmul(gub[:], ones_row[:], gu_row[:], start=True, stop=True)

            # ---------------- A branch ----------------------------------------
            act = pp.tile([128, C], F32, tag="act")
            nc.vector.tensor_scalar(act[:], denf, 0.0, None, OP.is_gt)
            oma = wp.tile([128, C], F32, tag="oma")
            nc.vector.tensor_scalar(oma[:], act[:], -1.0, 1.0, OP.mult, OP.add)
            nc.vector.tensor_add(oma[:], denf, oma[:])       # den + (1-active)
            rden = wp.tile([128, C], F32, tag="rden")
            nc.vector.reciprocal(rden[:], oma[:])
            va = pp.tile([128, C], F32, tag="va")
            nc.vector.tensor_mul(va[:], numf, act[:])
            nc.vector.tensor_mul(va[:], va[:], rden[:])

            vaT = pp.tile([128, CT, 128], F32R, tag="vaT")
            actT = pp.tile([128, CT, 128], F32R, tag="actT")
            for cb in range(CT):
                transpose_to(vaT[:, cb, :], va[:, 128 * cb : 128 * (cb + 1)])
                transpose_to(actT[:, cb, :], act[:, 128 * cb : 128 * (cb + 1)])

            psD_cm = tc.tile_pool(name="psD", bufs=1, space="PSUM")
            psD = psD_cm.__enter__()
            numA = psD.tile([128, C], F32, tag="numA")
            denA = psD.tile([128, C], F32, tag="denA")
            for kc in range(CT):
                st, sp = (kc == 0), (kc == CT - 1)
                nc.tensor.matmul(numA[:], vaT[:, kc, :], Eexp[:, kc, :], start=st, stop=sp)
                nc.tensor.matmul(denA[:], actT[:, kc, :], Eexp[:, kc, :], start=st, stop=sp)
            rdA = wp.tile([128, C], F32, tag="rdA")
            nc.vector.reciprocal(rdA[:], denA[:])
            A_sb = pp.tile([128, C], F32, tag="A_sb")
            nc.vector.tensor_mul(A_sb[:], numA[:], rdA[:])
            nc.sync.dma_start(a_out[:], A_sb[:])
            AT = pp.tile([128, CT, 128], F32R, tag="AT")
            for cb in range(CT):
                transpose_to(AT[:, cb, :], A_sb[:, 128 * cb : 128 * (cb + 1)])
            psD_cm.__exit__(None, None, None)

            # ---------------- Phase E: finish Y = YB + YA ----------------------
            for kc in range(CT):
                nc.tensor.matmul(yps[:], AT[:, kc, :], sm1T[:, kc, :],
                                 start=False, stop=(kc == CT - 1))
            yc = pp.tile([128, ESH], F32, tag="yc")
            nc.vector.tensor_scalar(yc[:], yps[:], 1e-8, 1.0, OP.max, OP.min)
            nc.vector.tensor_mul(yc[:], yc[:], c2b[:])
            nc.vector.tensor_add(yc[:], yc[:], gub[:])
            nc.sync.dma_start(y_out[:], yc[:])
            psE_cm.__exit__(None, None, None)
            tpp_cm.__exit__(None, None, None)

    _split_excess_waits(nc)
    return nc


_NC = None


def _get_nc():
    global _NC
    if _NC is None:
        _NC = build_nc()
    return _NC


def kernel(exer_idx, mask, scores, exer_matrix, conc_matrix, pote_matrix,
           lambd, guess, slide, adj):
    exer_idx = np.asarray(exer_idx)
    mask = np.asarray(mask)
    scores = np.asarray(scores, dtype=np.float32)
    exer_matrix = np.asarray(exer_matrix, dtype=np.float32)
    conc_matrix = np.asarray(conc_matrix, dtype=np.float32)
    pote_matrix = np.asarray(pote_matrix, dtype=np.float32)
    lambd = np.asarray(lambd, dtype=np.float32)
    guess = np.asarray(guess, dtype=np.float32)
    slide = np.asarray(slide, dtype=np.float32)
    adj = np.asarray(adj, dtype=np.float32)

    import ml_dtypes
    iota_bf = np.broadcast_to(
        np.arange(128, dtype=np.float32), (128, 128)).astype(ml_dtypes.bfloat16)
    ident = np.eye(128, dtype=np.float32)

    in_maps = []
    for c in range(NCORES):
        es = slice(ESH * c, ESH * (c + 1))
        ns = slice(NSH * c, NSH * (c + 1))
        in_maps.append({
            "exer_sh": np.ascontiguousarray(exer_matrix[es]),
            "conc": conc_matrix,
            "pote": pote_matrix,
            "adj_sh": np.ascontiguousarray(adj[es]),
            "q_bf": (exer_idx[ns] >> 5).astype(ml_dtypes.bfloat16),
            "r_bf": (exer_idx[ns] & 31).astype(ml_dtypes.bfloat16),
            "mask_bf": mask[ns].astype(ml_dtypes.bfloat16),
            "scores_bf": scores[ns].astype(ml_dtypes.bfloat16),
            "lambd_col": np.ascontiguousarray(
                lambd[0, es].reshape(ET, 128).T),
            "slide_sh": np.ascontiguousarray(slide[:, es]),
            "guess_sh": np.ascontiguousarray(guess[:, es]),
            "iota_bf": iota_bf,
            "ident": ident,
        })

    res = run_bass_kernel_spmd(_get_nc(), in_maps, list(range(NCORES)))
    A = res.results[0]["A_out"]
    Y = np.concatenate([res.results[c]["Y_sh"] for c in range(NCORES)], axis=1)
    if DEBUG:
        kernel.debug_results = res.results
    return A, Y
